# revision 29
# baseline (speedup 1.0000x reference)
"""Self-attention (SAGAN-style) Bass kernel for one TRN2 chip (8 NeuronCores).

Reference computation (B=4, H=W=64, C=256, D=32, N=H*W=4096):
    xf = x.reshape(B, N, C)
    k = xf @ Wk + bk; q = xf @ Wq + bq; v = xf @ Wv + bv
    energy = q @ k^T            # [B, N, N]
    attn = softmax(energy, -1)
    feat = attn @ v
    out = gamma * (feat @ Wo + bo) + xf

Sharding: core i handles batch b=i//2, query-row half h=i%2 (2048 rows).
k/v are computed over the full 4096 rows on every core (replicated, cheap).

Host-side exact folds (no device cost):
  - bk drops out of softmax (adds a per-row constant to energy).
  - v-bias: attn rows sum to 1 so attn@(v+bv) = attn@v + bv; fold
    gamma*(bv@Wo + bo) into the residual rows; gamma scales Wo itself.
  - bq enters energy as c_m = (k_m . bq), i.e. exp(S + c_m) =
    exp(S)*exp(c_m): a per-KEY scale that can be folded into v and the
    softmax-denominator ones column.  c_m = x_m . (Wk@bq) is computed on
    device as an extra wv column (the v projection already produces
    m-rows on partitions), so no transposes are needed.  When bq == 0
    (this problem's inputs) the whole path is compiled out.

Device pipeline per core, all PE operands bf16 (f32 PSUM accumulate):
  prologue: chunked xT DMA overlapped with q/k/v projections.
    qT [32,2048] replicated to partition groups 0/32/64.
    kT4: m-tile j at partition group 32*(slot) and col 128*(window).
    v4 [128, 33*32]: col 32 of each block = 1 (softmax denominator).
  main loop (s = 4 superblocks of 512 q-cols; 11 windows of 3|2 m-tiles):
    S^T: per window, one matmul per m-tile, 4x-row-packed
         (tile_position (32r,0), K=32) -> PSUM [128, 512*nw]
    exp: single ScalarE activation [128, 1536] PSUM->SBUF bf16
    PV:  2x-packed 64x64 tiles (0,0)+(64,64): U partials accumulate in
         partitions 0:33 / 64:97 of ONE psum bank across all 32 m-tiles
         (row 32 = column sums r, via the v4 ones column).
  epilogue per superblock: usb = (U0+U1) bf16, out-proj vs Wo_aug
  (col 256 carries r through), out = O*(1/r) + xr  (residual, f32).
ScalarE exp (~67us) is the designed critical path; PE work fits under it.
"""
import numpy as np
from contextlib import ExitStack

import concourse.bass as bass
import concourse.bacc as bacc
import concourse.tile as tile
from concourse import mybir
from concourse import bass_utils

F32 = mybir.dt.float32
BF16 = mybir.dt.bfloat16

B, HH, WW, C = 4, 64, 64, 256
N = HH * WW          # 4096 key/value rows
D = 32               # head dim
NCORES = 8
QSH = N // 2         # 2048 query rows per core
SBW = 512            # superblock width (q columns)
NSB = QSH // SBW     # 4 superblocks
NMT = N // 128       # 32 m-tiles
NW = [3] * 10 + [2]  # m-tiles per window (sum = 32)
ts = bass.ts

# m-tile j -> (window w, slot r); slot r uses PE row group 32r
_J2WR = {}
_jb = 0
for _w, _nw in enumerate(NW):
    for _r in range(_nw):
        _J2WR[_jb + _r] = (_w, _r)
    _jb += _nw


def build_graph(with_bq=False):
    """Build and compile the per-core Bass graph (identical on all cores)."""
    nc = bacc.Bacc("TRN2", target_bir_lowering=False, debug=False)

    xT_d = nc.dram_tensor("xT", [C, N], BF16, kind="ExternalInput").ap()
    xr_d = nc.dram_tensor("xr", [QSH, C], F32, kind="ExternalInput").ap()
    wk_d = nc.dram_tensor("wk", [C, D], BF16, kind="ExternalInput").ap()
    wq_d = nc.dram_tensor("wq", [C, 3 * D], BF16, kind="ExternalInput").ap()
    NV = 33 if with_bq else 32  # wv gets an extra Wk@bq column when bq != 0
    wv_d = nc.dram_tensor("wv", [C, NV], BF16, kind="ExternalInput").ap()
    wo_d = nc.dram_tensor("wo", [33, C + 2], BF16, kind="ExternalInput").ap()
    out_d = nc.dram_tensor("out", [QSH, C], F32, kind="ExternalOutput").ap()

    ExpF = mybir.ActivationFunctionType.Exp
    Amul = mybir.AluOpType.mult
    Aadd = mybir.AluOpType.add

    with tile.TileContext(nc) as tc, ExitStack() as ctx:
        persist = ctx.enter_context(tc.tile_pool(name="persist", bufs=1))
        stp = ctx.enter_context(tc.tile_pool(name="stp", bufs=2, space="PSUM"))
        uTp = ctx.enter_context(tc.tile_pool(name="uTp", bufs=1, space="PSUM"))
        miscp = ctx.enter_context(
            tc.tile_pool(name="miscp", bufs=1, space="PSUM"))
        expp = ctx.enter_context(tc.tile_pool(name="expp", bufs=3))
        smallp = ctx.enter_context(tc.tile_pool(name="smallp", bufs=2))
        outsb = ctx.enter_context(tc.tile_pool(name="outsb", bufs=3))

        # ---- persistent SBUF tensors ----
        # xT in 4 column chunks x 2 channel halves (separate tiles so the
        # projections for chunk t only depend on chunk t's DMA).
        xt = [[persist.tile([128, 1024], BF16, name=f"xt{t}_{hf}")
               for hf in range(2)] for t in range(4)]
        xr_sb = persist.tile([128, 16 * C], F32)  # residual row-tile t at 256t
        ost_all = persist.tile([128, 16 * C], F32)  # output staging, same map
        wk_sb = persist.tile([128, 64], BF16)
        wq_sb = persist.tile([128, 192], BF16)
        wv_sb = persist.tile([128, 2 * NV], BF16)
        wo_sb = persist.tile([33, C + 2], BF16)
        qT4 = persist.tile([128, QSH], BF16)      # rows 32r:32r+32, r=0..2
        # kT per window (tile-granular deps: S^T(w) waits only its own
        # scatter DMAs, not the whole k projection)
        kT4w = [persist.tile([128, 128], BF16, name=f"kT4w{w}")
                for w in range(len(NW))]
        # v4 split per window so PV(w) only depends on its own m-tiles'
        # v-projection (issued one window ahead inside the main loop)
        v4w = [persist.tile([128, 33 * nw], BF16, name=f"v4w{w}")
               for w, nw in enumerate(NW)]
        dummy = persist.tile([1, 1], F32)
        scratch = persist.tile([128, 512], BF16)
        if with_bq:
            kbq_sb = persist.tile([128, NMT], F32)
            ekbq = persist.tile([128, NMT], F32)

        # preload the exp table set while the prologue runs
        nc.vector.memset(dummy[:], 0.0)
        nc.scalar.activation(dummy[:], dummy[:], ExpF)

        # HAM pre-warm: ~5us of dummy matmuls while the input DMAs run, so
        # the PE hits the projections (and main loop) at 2.4 GHz.  Any PE
        # idle gap re-throttles the clock to 1.2 GHz, so start immediately
        # on data with no DMA dependency.
        nc.vector.memset(scratch[:], 0.0)
        for _ in range(12):
            dm = stp.tile([128, SBW], F32, tag="st")
            nc.tensor.matmul(dm[:], scratch[:, 0:128], scratch[:],
                             start=True, stop=True)

        # ---- input DMAs, spread over 4 engine queues ----
        # (xT chunks gate the projections: issue them first; xr is only
        # needed by the epilogues: issue it last)
        # DMA triggers BLOCK the issuing engine for the whole transfer, so
        # only sync+gpsimd (otherwise idle) carry DMAs; scalar runs exps
        # only.  xt chunk 0 + wq/wk first: they gate the q/k projections.
        qs = [nc.sync, nc.gpsimd]
        # inputs ride 3 queues (scalar's FIFO is free this early); wv
        # before xt2/3 -- the v projection starts at ~15us
        nc.sync.dma_start(xt[0][0][:], xT_d[0:128, ts(0, 1024)])
        nc.gpsimd.dma_start(xt[0][1][:], xT_d[128:256, ts(0, 1024)])
        nc.scalar.dma_start(wq_sb[:, 0:96], wq_d[0:128, :])
        nc.sync.dma_start(wq_sb[:, 96:192], wq_d[128:256, :])
        nc.gpsimd.dma_start(wk_sb[:, 0:32], wk_d[0:128, :])
        nc.scalar.dma_start(wk_sb[:, 32:64], wk_d[128:256, :])
        nc.sync.dma_start(xt[1][0][:], xT_d[0:128, ts(1, 1024)])
        nc.gpsimd.dma_start(xt[1][1][:], xT_d[128:256, ts(1, 1024)])
        nc.scalar.dma_start(wv_sb[:, 0:NV], wv_d[0:128, :])
        nc.sync.dma_start(wv_sb[:, NV:2 * NV], wv_d[128:256, :])
        nc.gpsimd.dma_start(xt[2][0][:], xT_d[0:128, ts(2, 1024)])
        nc.scalar.dma_start(xt[2][1][:], xT_d[128:256, ts(2, 1024)])
        nc.sync.dma_start(xt[3][0][:], xT_d[0:128, ts(3, 1024)])
        nc.gpsimd.dma_start(xt[3][1][:], xT_d[128:256, ts(3, 1024)])
        nc.scalar.dma_start(wo_sb[:], wo_d)
        if not with_bq:
            for vt in v4w:  # col 32 of each block stays 1
                nc.gpsimd.memset(vt[:], 1.0)

        # ---- projections ----
        # PSUM->SBUF casts alternate between VectorE and ScalarE; GpSimd
        # (no PSUM port) handles the SBUF->SBUF qT replication.
        cast_engs = [nc.vector, nc.scalar]
        cast_i = 0

        def cast(out, in_):
            nonlocal cast_i
            e = cast_engs[cast_i % 2]
            cast_i += 1
            if e is nc.scalar:
                e.copy(out, in_)
            else:
                e.tensor_copy(out, in_)

        # q/k projection PSUM tiles round-robin over every ring that is
        # still unused this early (depth-4 pipelining keeps the PE dense;
        # a 2-deep ring starves it on cast semaphores and re-throttles HAM)
        rings = [(stp, "st"), (stp, "st"), (miscp, "mp"), (uTp, "uT")]
        ring_i = 0

        def proj_tile(shape):
            nonlocal ring_i
            pool, tag = rings[ring_i % 4]
            ring_i += 1
            return pool.tile(shape, F32, name="pt", tag=tag)

        # ---- attention main loop state ----
        # Software-pipelined issue order: the PE queue is FIFO, so S^T of
        # window i+2 is issued BEFORE PV of window i -- PV(i) waits ~1.5us
        # on exp(i) and everything behind it in the queue would stall.
        # PV runs as solo K=128 matmuls: solo bf16 matmuls stream 2
        # cols/cycle (double-pumped), while concurrently row-packed ones
        # drop to 1 col/cycle, so packing loses for full-K PV.
        uT = uTp.tile([128, SBW], F32, name="uT", tag="uT")
        seq = [(s, w) for s in range(NSB) for w in range(len(NW))]
        jstart = {w: sum(NW[:w]) for w in range(len(NW))}

        def issue_st(s, w):
            nw = NW[w]
            st = stp.tile([128, SBW * nw], F32, tag="st")
            for r in range(nw):
                nc.tensor.matmul(st[:, ts(r, SBW)],
                                 kT4w[w][32 * r:32 * r + 32, :],
                                 qT4[32 * r:32 * r + 32, ts(s, SBW)],
                                 start=True, stop=True,
                                 tile_position=(32 * r, 0))
            return st

        # q projection: wq3 = [Wq|Wq|Wq] host-side, so one matmul pair per
        # 512-block writes all 3 PE-row-group replicas of qT at once
        for t in range(2):
            x0, x1 = xt[t]
            for half in range(2):
                pq = proj_tile([96, SBW])
                nc.tensor.matmul(pq[0:96, :], wq_sb[:, 0:96],
                                 x0[:, ts(half, SBW)],
                                 start=True, stop=False)
                nc.tensor.matmul(pq[0:96, :], wq_sb[:, 96:192],
                                 x1[:, ts(half, SBW)],
                                 start=False, stop=True)
                nt = 2 * t + half
                cast(qT4[0:96, ts(nt, SBW)], pq[0:96, :])
        # k projection: column-tiled matmuls (tile_position (0,32r)) land
        # each m-tile's k^T directly in partition group r of one PSUM tile;
        # a single [96,128] cast per window fills kT4w -- no scatter DMAs
        def issue_kproj(w):
            nw = NW[w]
            pk = proj_tile([128, 128])
            for r in range(nw):
                j = jstart[w] + r
                x0, x1 = xt[j // 8]
                lo = 128 * (j % 8)
                nc.tensor.matmul(pk[32 * r:32 * r + 32, 0:128],
                                 wk_sb[:, 0:32], x0[:, lo:lo + 128],
                                 start=True, stop=False,
                                 skip_group_check=True,
                                 tile_position=(0, 32 * r))
                nc.tensor.matmul(pk[32 * r:32 * r + 32, 0:128],
                                 wk_sb[:, 32:64], x1[:, lo:lo + 128],
                                 start=False, stop=True,
                                 skip_group_check=True,
                                 tile_position=(0, 32 * r))
            cast(kT4w[w][0:32 * nw, :], pk[0:32 * nw, 0:128])

        def issue_vproj(w):
            # v projection for window w's m-tiles (xT chunk stationary, wv
            # moving).  Issued one window ahead of its PV consumer: these
            # matmuls fill the PE's exp-wait bubble inside each window.
            nw = NW[w]
            for r in range(nw):
                j = jstart[w] + r
                x0, x1 = xt[j // 8]
                pv = proj_tile([128, NV])
                nc.tensor.matmul(pv[:], x0[:, ts(j % 8, 128)],
                                 wv_sb[:, 0:NV], start=True, stop=False)
                nc.tensor.matmul(pv[:], x1[:, ts(j % 8, 128)],
                                 wv_sb[:, NV:2 * NV],
                                 start=False, stop=True)
                cast(v4w[w][:, 33 * r:33 * r + 32], pv[:, 0:32])
                if with_bq:
                    nc.vector.tensor_copy(kbq_sb[:, j:j + 1], pv[:, 32:33])
            if with_bq:
                # exact bq handling: scale v rows (and the ones column) by
                # exp(k_m . bq) so softmax num/denominator pick it up
                jb = jstart[w]
                nc.scalar.activation(ekbq[:, jb:jb + nw],
                                     kbq_sb[:, jb:jb + nw], ExpF)
                for r in range(nw):
                    j = jb + r
                    nc.vector.tensor_scalar(
                        v4w[w][:, 33 * r:33 * r + 32],
                        v4w[w][:, 33 * r:33 * r + 32],
                        ekbq[:, j:j + 1], None, op0=Amul)
                    nc.vector.tensor_copy(
                        v4w[w][:, 33 * r + 32:33 * r + 33],
                        ekbq[:, j:j + 1])

        # issue order tuned so exp(0) fires as early as possible: k/v for
        # windows 0-1, S^T(0)/S^T(1) immediately, then the remaining
        # windows' k/v projections fill the PE while the exp chain starts
        issue_kproj(0)
        issue_kproj(1)
        sts = {0: issue_st(*seq[0]), 1: issue_st(*seq[1])}
        issue_vproj(0)
        issue_vproj(1)
        for w in range(2, len(NW)):
            issue_kproj(w)
            if w < 4:  # pre-issue S^T(2),S^T(3): exp(2) must not wait for
                sts[w] = issue_st(*seq[w])  # the whole projection phase
            issue_vproj(w)
        for t in range(16):
            qs[t % 2].dma_start(xr_sb[:, ts(t, C)], xr_d[ts(t, 128), :])
        for i, (s, w) in enumerate(seq):
            nw = NW[w]
            st = sts.pop(i)
            ex = expp.tile([128, SBW * nw], BF16)
            nc.scalar.activation(ex[:], st[:], ExpF)
            if i + 2 < len(seq) and i + 2 not in sts:
                sts[i + 2] = issue_st(*seq[i + 2])
            for r in range(nw):
                j = jstart[w] + r
                nc.tensor.matmul(uT[0:33, :], v4w[w][:, 33 * r:33 * r + 33],
                                 ex[:, ts(r, SBW)],
                                 start=(j == 0), stop=(j == NMT - 1),
                                 skip_group_check=True)
            if w != len(NW) - 1:
                continue

            # ---- output projection for superblock s ----
            usb = smallp.tile([33, SBW], BF16, tag="usb")
            nc.vector.tensor_copy(usb[:], uT[0:33, :])
            for qb in range(SBW // 128):
                if s == NSB - 1:
                    o_ps = stp.tile([128, C + 2], F32, tag="st")
                else:
                    o_ps = miscp.tile([128, C + 2], F32, tag="mp")
                nc.tensor.matmul(o_ps[:], usb[:, ts(qb, 128)], wo_sb[:],
                                 start=True, stop=True)
                recip = smallp.tile([128, 1], F32, tag="recip")
                nc.vector.reciprocal(recip[:], o_ps[:, C:C + 1])
                qi = s * (SBW // 128) + qb
                nc.vector.scalar_tensor_tensor(
                    ost_all[:, ts(qi, C)], o_ps[:, 0:C], recip[:],
                    xr_sb[:, ts(qi, C)], op0=Amul, op1=Aadd)
            dst = out_d[ts(s, SBW), :].rearrange("(qb p) c -> p qb c", p=128)
            osrc = ost_all[:, ts(s, 4 * C)].rearrange("p (qb c) -> p qb c", c=C)
            qs[s % 2].dma_start(dst, osrc)

    nc.compile()
    return nc


_NC_CACHE = {}


def _get_nc(with_bq=False):
    if with_bq not in _NC_CACHE:
        _NC_CACHE[with_bq] = build_graph(with_bq)
    return _NC_CACHE[with_bq]


def _bf16(a):
    import ml_dtypes
    return np.ascontiguousarray(np.asarray(a, dtype=np.float32)
                                .astype(ml_dtypes.bfloat16))


def make_in_maps(x, Wk, bk, Wq, bq, Wv, bv, Wo, bo, gamma):
    """Host-side sharding + exact bias/gamma folding."""
    f32 = np.float32
    xf = np.ascontiguousarray(x, dtype=f32).reshape(B, N, C)
    Wk = np.asarray(Wk, dtype=f32)
    Wq = np.asarray(Wq, dtype=f32)
    Wv = np.asarray(Wv, dtype=f32)
    Wo = np.asarray(Wo, dtype=f32)
    bq = np.asarray(bq, dtype=f32)
    bv = np.asarray(bv, dtype=f32)
    bo = np.asarray(bo, dtype=f32)
    g = np.asarray(gamma, dtype=f32)[0]

    with_bq = bool(np.any(bq != 0.0))
    if with_bq:
        wv_in = np.concatenate([Wv, (Wk @ bq)[:, None]], axis=1)  # [C, 33]
    else:
        wv_in = Wv
    wo_aug = np.zeros((33, C + 2), dtype=f32)
    wo_aug[0:32, 0:C] = g * Wo
    wo_aug[32, C] = 1.0
    xr_bias = (g * (bv @ Wo + bo)).astype(f32)  # folded into the residual

    wk_b = _bf16(Wk)
    wq_b = _bf16(np.concatenate([Wq, Wq, Wq], axis=1))  # 3 replica groups
    wv_b = _bf16(wv_in)
    wo_b = _bf16(wo_aug)

    in_maps = []
    for i in range(NCORES):
        b, h = divmod(i, 2)
        own = xf[b, h * QSH:(h + 1) * QSH]
        other = xf[b, (1 - h) * QSH:(2 - h) * QSH]
        xT = np.concatenate([own, other], axis=0).T  # [C, N], own cols first
        in_maps.append({
            "xT": _bf16(xT),
            "xr": np.ascontiguousarray(own + xr_bias),
            "wk": wk_b,
            "wq": wq_b,
            "wv": wv_b,
            "wo": wo_b,
        })
    return in_maps, with_bq


def gather_out(results, x_dtype):
    out = np.empty((B, N, C), dtype=np.float32)
    for i in range(NCORES):
        b, h = divmod(i, 2)
        out[b, h * QSH:(h + 1) * QSH] = results[i]["out"]
    return out.reshape(B, HH, WW, C).astype(x_dtype, copy=False)


def kernel(x, Wk, bk, Wq, bq, Wv, bv, Wo, bo, gamma, **run_kwargs):
    in_maps, with_bq = make_in_maps(x, Wk, bk, Wq, bq, Wv, bv, Wo, bo, gamma)
    nc = _get_nc(with_bq)
    res = bass_utils.run_bass_kernel_spmd(
        nc, in_maps, core_ids=list(range(NCORES)), **run_kwargs
    )
    out = gather_out(res.results, np.asarray(x).dtype)
    if run_kwargs:
        return out, res
    return out


# revision 31
# speedup vs baseline: 1.0417x; 1.0417x over previous
"""Self-attention (SAGAN-style) Bass kernel for one TRN2 chip (8 NeuronCores).

Reference computation (B=4, H=W=64, C=256, D=32, N=H*W=4096):
    xf = x.reshape(B, N, C)
    k = xf @ Wk + bk; q = xf @ Wq + bq; v = xf @ Wv + bv
    energy = q @ k^T            # [B, N, N]
    attn = softmax(energy, -1)
    feat = attn @ v
    out = gamma * (feat @ Wo + bo) + xf

Sharding: core i handles batch b=i//2, query-row half h=i%2 (2048 rows).
k/v are computed over the full 4096 rows on every core (replicated, cheap).

Host-side exact folds (no device cost):
  - bk drops out of softmax (adds a per-row constant to energy).
  - v-bias: attn rows sum to 1 so attn@(v+bv) = attn@v + bv; fold
    gamma*(bv@Wo + bo) into the residual rows; gamma scales Wo itself.
  - bq enters energy as c_m = (k_m . bq), i.e. exp(S + c_m) =
    exp(S)*exp(c_m): a per-KEY scale that can be folded into v and the
    softmax-denominator ones column.  c_m = x_m . (Wk@bq) is computed on
    device as an extra wv column (the v projection already produces
    m-rows on partitions), so no transposes are needed.  When bq == 0
    (this problem's inputs) the whole path is compiled out.

Device pipeline per core, all PE operands bf16 (f32 PSUM accumulate):
  prologue: chunked xT DMA overlapped with q/k/v projections.
    qT [32,2048] replicated to partition groups 0/32/64.
    kT4: m-tile j at partition group 32*(slot) and col 128*(window).
    v4 [128, 33*32]: col 32 of each block = 1 (softmax denominator).
  main loop (s = 4 superblocks of 512 q-cols; 11 windows of 3|2 m-tiles):
    S^T: per window, one matmul per m-tile, 4x-row-packed
         (tile_position (32r,0), K=32) -> PSUM [128, 512*nw]
    exp: single ScalarE activation [128, 1536] PSUM->SBUF bf16
    PV:  2x-packed 64x64 tiles (0,0)+(64,64): U partials accumulate in
         partitions 0:33 / 64:97 of ONE psum bank across all 32 m-tiles
         (row 32 = column sums r, via the v4 ones column).
  epilogue per superblock: usb = (U0+U1) bf16, out-proj vs Wo_aug
  (col 256 carries r through), out = O*(1/r) + xr  (residual, f32).
ScalarE exp (~67us) is the designed critical path; PE work fits under it.
"""
import numpy as np
from contextlib import ExitStack

import concourse.bass as bass
import concourse.bacc as bacc
import concourse.tile as tile
from concourse import mybir
from concourse import bass_utils

F32 = mybir.dt.float32
BF16 = mybir.dt.bfloat16

B, HH, WW, C = 4, 64, 64, 256
N = HH * WW          # 4096 key/value rows
D = 32               # head dim
NCORES = 8
QSH = N // 2         # 2048 query rows per core
SBW = 512            # superblock width (q columns)
NSB = QSH // SBW     # 4 superblocks
NMT = N // 128       # 32 m-tiles
NW = [3] * 10 + [2]  # m-tiles per window (sum = 32)
ts = bass.ts

# m-tile j -> (window w, slot r); slot r uses PE row group 32r
_J2WR = {}
_jb = 0
for _w, _nw in enumerate(NW):
    for _r in range(_nw):
        _J2WR[_jb + _r] = (_w, _r)
    _jb += _nw


def build_graph(with_bq=False):
    """Build and compile the per-core Bass graph (identical on all cores)."""
    nc = bacc.Bacc("TRN2", target_bir_lowering=False, debug=False)

    xT_d = nc.dram_tensor("xT", [C, N], BF16, kind="ExternalInput").ap()
    xr_d = nc.dram_tensor("xr", [QSH, C], F32, kind="ExternalInput").ap()
    wk_d = nc.dram_tensor("wk", [C, D], BF16, kind="ExternalInput").ap()
    wq_d = nc.dram_tensor("wq", [C, 3 * D], BF16, kind="ExternalInput").ap()
    NV = 33 if with_bq else 32  # wv gets an extra Wk@bq column when bq != 0
    wv_d = nc.dram_tensor("wv", [C, NV], BF16, kind="ExternalInput").ap()
    wo_d = nc.dram_tensor("wo", [33, C + 2], BF16, kind="ExternalInput").ap()
    out_d = nc.dram_tensor("out", [QSH, C], F32, kind="ExternalOutput").ap()

    ExpF = mybir.ActivationFunctionType.Exp
    Amul = mybir.AluOpType.mult
    Aadd = mybir.AluOpType.add

    with tile.TileContext(nc) as tc, ExitStack() as ctx:
        persist = ctx.enter_context(tc.tile_pool(name="persist", bufs=1))
        stp = ctx.enter_context(tc.tile_pool(name="stp", bufs=2, space="PSUM"))
        uTp = ctx.enter_context(tc.tile_pool(name="uTp", bufs=1, space="PSUM"))
        miscp = ctx.enter_context(
            tc.tile_pool(name="miscp", bufs=1, space="PSUM"))
        expp = ctx.enter_context(tc.tile_pool(name="expp", bufs=3))
        smallp = ctx.enter_context(tc.tile_pool(name="smallp", bufs=2))
        outsb = ctx.enter_context(tc.tile_pool(name="outsb", bufs=3))

        # ---- persistent SBUF tensors ----
        # xT in 4 column chunks x 2 channel halves (separate tiles so the
        # projections for chunk t only depend on chunk t's DMA).
        xt = [[persist.tile([128, 1024], BF16, name=f"xt{t}_{hf}")
               for hf in range(2)] for t in range(4)]
        xr_sb = persist.tile([128, 16 * C], F32)  # residual row-tile t at 256t
        ost_all = persist.tile([128, 16 * C], F32)  # output staging, same map
        wk_sb = persist.tile([128, 64], BF16)
        wq_sb = persist.tile([128, 192], BF16)
        wv_sb = persist.tile([128, 2 * NV], BF16)
        wo_sb = persist.tile([33, C + 2], BF16)
        qT4 = persist.tile([128, QSH], BF16)      # rows 32r:32r+32, r=0..2
        # kT per window (tile-granular deps: S^T(w) waits only its own
        # scatter DMAs, not the whole k projection)
        kT4w = [persist.tile([128, 128], BF16, name=f"kT4w{w}")
                for w in range(len(NW))]
        # v4 split per window so PV(w) only depends on its own m-tiles'
        # v-projection (issued one window ahead inside the main loop)
        v4w = [persist.tile([128, 33 * nw], BF16, name=f"v4w{w}")
               for w, nw in enumerate(NW)]
        dummy = persist.tile([1, 1], F32)
        scratch = persist.tile([128, 512], BF16)
        if with_bq:
            kbq_sb = persist.tile([128, NMT], F32)
            ekbq = persist.tile([128, NMT], F32)

        # preload the exp table set while the prologue runs
        nc.vector.memset(dummy[:], 0.0)
        nc.scalar.activation(dummy[:], dummy[:], ExpF)

        # HAM pre-warm: ~5us of dummy matmuls while the input DMAs run, so
        # the PE hits the projections (and main loop) at 2.4 GHz.  Any PE
        # idle gap re-throttles the clock to 1.2 GHz, so start immediately
        # on data with no DMA dependency.
        nc.vector.memset(scratch[:], 0.0)
        for _ in range(12):
            dm = stp.tile([128, SBW], F32, tag="st")
            nc.tensor.matmul(dm[:], scratch[:, 0:128], scratch[:],
                             start=True, stop=True)

        # ---- input DMAs, spread over 4 engine queues ----
        # (xT chunks gate the projections: issue them first; xr is only
        # needed by the epilogues: issue it last)
        # DMA triggers BLOCK the issuing engine for the whole transfer, so
        # only sync+gpsimd (otherwise idle) carry DMAs; scalar runs exps
        # only.  xt chunk 0 + wq/wk first: they gate the q/k projections.
        qs = [nc.sync, nc.gpsimd]
        # inputs ride 3 queues (scalar's FIFO is free this early); wv
        # before xt2/3 -- the v projection starts at ~15us
        nc.sync.dma_start(xt[0][0][:], xT_d[0:128, ts(0, 1024)])
        nc.gpsimd.dma_start(xt[0][1][:], xT_d[128:256, ts(0, 1024)])
        nc.scalar.dma_start(wq_sb[:, 0:96], wq_d[0:128, :])
        nc.sync.dma_start(wq_sb[:, 96:192], wq_d[128:256, :])
        nc.gpsimd.dma_start(wk_sb[:, 0:32], wk_d[0:128, :])
        nc.scalar.dma_start(wk_sb[:, 32:64], wk_d[128:256, :])
        nc.sync.dma_start(xt[1][0][:], xT_d[0:128, ts(1, 1024)])
        nc.gpsimd.dma_start(xt[1][1][:], xT_d[128:256, ts(1, 1024)])
        nc.scalar.dma_start(wv_sb[:, 0:NV], wv_d[0:128, :])
        nc.sync.dma_start(wv_sb[:, NV:2 * NV], wv_d[128:256, :])
        nc.gpsimd.dma_start(xt[2][0][:], xT_d[0:128, ts(2, 1024)])
        nc.scalar.dma_start(xt[2][1][:], xT_d[128:256, ts(2, 1024)])
        nc.sync.dma_start(xt[3][0][:], xT_d[0:128, ts(3, 1024)])
        nc.gpsimd.dma_start(xt[3][1][:], xT_d[128:256, ts(3, 1024)])
        nc.scalar.dma_start(wo_sb[:], wo_d)
        if not with_bq:
            for vt in v4w:  # col 32 of each block stays 1
                nc.gpsimd.memset(vt[:], 1.0)

        # ---- projections ----
        # PSUM->SBUF casts alternate between VectorE and ScalarE; GpSimd
        # (no PSUM port) handles the SBUF->SBUF qT replication.
        cast_engs = [nc.vector, nc.scalar]
        cast_i = 0

        def cast(out, in_):
            nonlocal cast_i
            e = cast_engs[cast_i % 2]
            cast_i += 1
            if e is nc.scalar:
                e.copy(out, in_)
            else:
                e.tensor_copy(out, in_)

        # q/k projection PSUM tiles round-robin over every ring that is
        # still unused this early (depth-4 pipelining keeps the PE dense;
        # a 2-deep ring starves it on cast semaphores and re-throttles HAM)
        rings = [(stp, "st"), (stp, "st"), (miscp, "mp"), (uTp, "uT")]
        ring_i = 0

        def proj_tile(shape):
            nonlocal ring_i
            pool, tag = rings[ring_i % 4]
            ring_i += 1
            return pool.tile(shape, F32, name="pt", tag=tag)

        # ---- attention main loop state ----
        # Software-pipelined issue order: the PE queue is FIFO, so S^T of
        # window i+2 is issued BEFORE PV of window i -- PV(i) waits ~1.5us
        # on exp(i) and everything behind it in the queue would stall.
        # PV runs as solo K=128 matmuls: solo bf16 matmuls stream 2
        # cols/cycle (double-pumped), while concurrently row-packed ones
        # drop to 1 col/cycle, so packing loses for full-K PV.
        uT = uTp.tile([128, SBW], F32, name="uT", tag="uT")
        seq = [(s, w) for s in range(NSB) for w in range(len(NW))]
        jstart = {w: sum(NW[:w]) for w in range(len(NW))}

        def issue_st(s, w):
            nw = NW[w]
            st = stp.tile([128, SBW * nw], F32, tag="st")
            for r in range(nw):
                nc.tensor.matmul(st[:, ts(r, SBW)],
                                 kT4w[w][32 * r:32 * r + 32, :],
                                 qT4[32 * r:32 * r + 32, ts(s, SBW)],
                                 start=True, stop=True,
                                 tile_position=(32 * r, 0))
            return st

        # q projection: wq3 = [Wq|Wq|Wq] host-side, so one matmul pair per
        # 512-block writes all 3 PE-row-group replicas of qT at once
        for t in range(2):
            x0, x1 = xt[t]
            for half in range(2):
                pq = proj_tile([96, SBW])
                nc.tensor.matmul(pq[0:96, :], wq_sb[:, 0:96],
                                 x0[:, ts(half, SBW)],
                                 start=True, stop=False)
                nc.tensor.matmul(pq[0:96, :], wq_sb[:, 96:192],
                                 x1[:, ts(half, SBW)],
                                 start=False, stop=True)
                nt = 2 * t + half
                cast(qT4[0:96, ts(nt, SBW)], pq[0:96, :])
        # k projection: column-tiled matmuls (tile_position (0,32r)) land
        # each m-tile's k^T directly in partition group r of one PSUM tile;
        # a single [96,128] cast per window fills kT4w -- no scatter DMAs
        def issue_kproj(w):
            nw = NW[w]
            pk = proj_tile([128, 128])
            for r in range(nw):
                j = jstart[w] + r
                x0, x1 = xt[j // 8]
                lo = 128 * (j % 8)
                nc.tensor.matmul(pk[32 * r:32 * r + 32, 0:128],
                                 wk_sb[:, 0:32], x0[:, lo:lo + 128],
                                 start=True, stop=False,
                                 skip_group_check=True,
                                 tile_position=(0, 32 * r))
                nc.tensor.matmul(pk[32 * r:32 * r + 32, 0:128],
                                 wk_sb[:, 32:64], x1[:, lo:lo + 128],
                                 start=False, stop=True,
                                 skip_group_check=True,
                                 tile_position=(0, 32 * r))
            cast(kT4w[w][0:32 * nw, :], pk[0:32 * nw, 0:128])

        def issue_vproj(w, inloop=False):
            # v projection for window w's m-tiles (xT chunk stationary, wv
            # moving).  In-loop issues use the mp ring only: the st ring
            # would create circular WARs with the S^T ping-pong.
            nw = NW[w]
            for r in range(nw):
                j = jstart[w] + r
                x0, x1 = xt[j // 8]
                if inloop:
                    pv = miscp.tile([128, NV], F32, name="pt", tag="mp")
                else:
                    pv = proj_tile([128, NV])
                nc.tensor.matmul(pv[:], x0[:, ts(j % 8, 128)],
                                 wv_sb[:, 0:NV], start=True, stop=False)
                nc.tensor.matmul(pv[:], x1[:, ts(j % 8, 128)],
                                 wv_sb[:, NV:2 * NV],
                                 start=False, stop=True)
                cast(v4w[w][:, 33 * r:33 * r + 32], pv[:, 0:32])
                if with_bq:
                    nc.vector.tensor_copy(kbq_sb[:, j:j + 1], pv[:, 32:33])
            if with_bq:
                # exact bq handling: scale v rows (and the ones column) by
                # exp(k_m . bq) so softmax num/denominator pick it up
                jb = jstart[w]
                nc.scalar.activation(ekbq[:, jb:jb + nw],
                                     kbq_sb[:, jb:jb + nw], ExpF)
                for r in range(nw):
                    j = jb + r
                    nc.vector.tensor_scalar(
                        v4w[w][:, 33 * r:33 * r + 32],
                        v4w[w][:, 33 * r:33 * r + 32],
                        ekbq[:, j:j + 1], None, op0=Amul)
                    nc.vector.tensor_copy(
                        v4w[w][:, 33 * r + 32:33 * r + 33],
                        ekbq[:, j:j + 1])

        # issue order tuned so exp(0) fires as early as possible: k/v for
        # windows 0-1, S^T(0)/S^T(1) immediately, then the remaining
        # windows' k/v projections fill the PE while the exp chain starts
        issue_kproj(0)
        issue_kproj(1)
        sts = {0: issue_st(*seq[0]), 1: issue_st(*seq[1])}
        issue_vproj(0)
        issue_vproj(1)
        for w in range(2, len(NW)):
            issue_kproj(w)
            if w < 4:  # pre-issue S^T(2),S^T(3): exp(2) must not wait for
                sts[w] = issue_st(*seq[w])  # the whole projection phase
            if w < 6:
                issue_vproj(w)
        for t in range(16):
            qs[t % 2].dma_start(xr_sb[:, ts(t, C)], xr_d[ts(t, 128), :])
        for i, (s, w) in enumerate(seq):
            nw = NW[w]
            st = sts.pop(i)
            ex = expp.tile([128, SBW * nw], BF16)
            nc.scalar.activation(ex[:], st[:], ExpF)
            if i + 2 < len(seq) and i + 2 not in sts:
                sts[i + 2] = issue_st(*seq[i + 2])
            if i + 6 < len(NW):  # v projections ride the s0 window slack
                issue_vproj(i + 6, inloop=True)
            for r in range(nw):
                j = jstart[w] + r
                nc.tensor.matmul(uT[0:33, :], v4w[w][:, 33 * r:33 * r + 33],
                                 ex[:, ts(r, SBW)],
                                 start=(j == 0), stop=(j == NMT - 1),
                                 skip_group_check=True)
            if w != len(NW) - 1:
                continue

            # ---- output projection for superblock s ----
            usb = smallp.tile([33, SBW], BF16, tag="usb")
            nc.vector.tensor_copy(usb[:], uT[0:33, :])
            for qb in range(SBW // 128):
                if s == NSB - 1:
                    o_ps = stp.tile([128, C + 2], F32, tag="st")
                else:
                    o_ps = miscp.tile([128, C + 2], F32, tag="mp")
                nc.tensor.matmul(o_ps[:], usb[:, ts(qb, 128)], wo_sb[:],
                                 start=True, stop=True)
                recip = smallp.tile([128, 1], F32, tag="recip")
                nc.vector.reciprocal(recip[:], o_ps[:, C:C + 1])
                qi = s * (SBW // 128) + qb
                nc.vector.scalar_tensor_tensor(
                    ost_all[:, ts(qi, C)], o_ps[:, 0:C], recip[:],
                    xr_sb[:, ts(qi, C)], op0=Amul, op1=Aadd)
                qs[qi % 2].dma_start(out_d[ts(qi, 128), :],
                                     ost_all[:, ts(qi, C)])

    nc.compile()
    return nc


_NC_CACHE = {}


def _get_nc(with_bq=False):
    if with_bq not in _NC_CACHE:
        _NC_CACHE[with_bq] = build_graph(with_bq)
    return _NC_CACHE[with_bq]


def _bf16(a):
    import ml_dtypes
    return np.ascontiguousarray(np.asarray(a, dtype=np.float32)
                                .astype(ml_dtypes.bfloat16))


def make_in_maps(x, Wk, bk, Wq, bq, Wv, bv, Wo, bo, gamma):
    """Host-side sharding + exact bias/gamma folding."""
    f32 = np.float32
    xf = np.ascontiguousarray(x, dtype=f32).reshape(B, N, C)
    Wk = np.asarray(Wk, dtype=f32)
    Wq = np.asarray(Wq, dtype=f32)
    Wv = np.asarray(Wv, dtype=f32)
    Wo = np.asarray(Wo, dtype=f32)
    bq = np.asarray(bq, dtype=f32)
    bv = np.asarray(bv, dtype=f32)
    bo = np.asarray(bo, dtype=f32)
    g = np.asarray(gamma, dtype=f32)[0]

    with_bq = bool(np.any(bq != 0.0))
    if with_bq:
        wv_in = np.concatenate([Wv, (Wk @ bq)[:, None]], axis=1)  # [C, 33]
    else:
        wv_in = Wv
    wo_aug = np.zeros((33, C + 2), dtype=f32)
    wo_aug[0:32, 0:C] = g * Wo
    wo_aug[32, C] = 1.0
    xr_bias = (g * (bv @ Wo + bo)).astype(f32)  # folded into the residual

    wk_b = _bf16(Wk)
    wq_b = _bf16(np.concatenate([Wq, Wq, Wq], axis=1))  # 3 replica groups
    wv_b = _bf16(wv_in)
    wo_b = _bf16(wo_aug)

    in_maps = []
    for i in range(NCORES):
        b, h = divmod(i, 2)
        own = xf[b, h * QSH:(h + 1) * QSH]
        other = xf[b, (1 - h) * QSH:(2 - h) * QSH]
        xT = np.concatenate([own, other], axis=0).T  # [C, N], own cols first
        in_maps.append({
            "xT": _bf16(xT),
            "xr": np.ascontiguousarray(own + xr_bias),
            "wk": wk_b,
            "wq": wq_b,
            "wv": wv_b,
            "wo": wo_b,
        })
    return in_maps, with_bq


def gather_out(results, x_dtype):
    out = np.empty((B, N, C), dtype=np.float32)
    for i in range(NCORES):
        b, h = divmod(i, 2)
        out[b, h * QSH:(h + 1) * QSH] = results[i]["out"]
    return out.reshape(B, HH, WW, C).astype(x_dtype, copy=False)


def kernel(x, Wk, bk, Wq, bq, Wv, bv, Wo, bo, gamma, **run_kwargs):
    in_maps, with_bq = make_in_maps(x, Wk, bk, Wq, bq, Wv, bv, Wo, bo, gamma)
    nc = _get_nc(with_bq)
    res = bass_utils.run_bass_kernel_spmd(
        nc, in_maps, core_ids=list(range(NCORES)), **run_kwargs
    )
    out = gather_out(res.results, np.asarray(x).dtype)
    if run_kwargs:
        return out, res
    return out
